# revision 17
# baseline (speedup 1.0000x reference)
"""Trainium2 Bass kernel for nn_CAM_62852551409742.

Math (reference):
  f = feats[:, :, 0, :]                               [R,B,T], R=4, B=512, T=150
  feat_n = feats.reshape(B, K)                        [B,K], K=600
  att[r,b,t,k] = tanh(a[r]*f[r,b,t] * feat_n[b,k])
  Hm = relu(att @ Wc[r].T + f*W[r])                   [R,B,T,32]
  attf = Hm @ Wh[r] + f                               [R,B,T]
  out = (ff @ W1.T + b1) @ W2.T + b2                  [B,1,7]

Key optimization: tanh of a *product* admits an odd-polynomial fit
tanh(z) ~= c1 z + c3 z^3 + c5 z^5 (runtime-LSQ-fit per rep on the actual
z distribution; rel err ~1e-6), which factorizes through the k-contraction:
  sum_k tanh(s*fn_k) Wc[c,k] = sum_j c_j s^j M_j[c],  M_j = fn^j @ Wc.T
so the 184M-element tanh tensor is never materialized. The f*W term is
folded into the M1 chain via wk = W/k1 (P row carries k1*f; k1 clamped).

Device work per core (B sharded 64/core, p == local batch):
  stage 1: M_j[p,(r,c)] = fn^j @ Wc.T (+ ones x wk fold into M1), col-tiled
           chains: j1@[0:64,0:128], j3@[64:128,0:128] (pos (0,64)),
           j5@[0:64,128:256].
  build Vt[q=j*4+r, b, (r,c)-block] block-diagonal [12,128] lhsT tiles:
        zero-filled by one DRAM DMA, 12 collapse-DMAs for the M blocks.
        P [q, b, t]: rows 0-3 streamed from DRAM (host-side k1*f), rows
        4-11 collapse-DMAs from DVE-computed k3*f^3, k5*f^5.
  stage 2: per b one [12,128]^T @ [12,150] matmul -> pre [128, 150];
           relu (DVE/ACT alternate, paces psum recycling) -> hm bf16
  final: U-trick (U[(rc),t,i] = Wh*Wx) 150 matmuls 4-way col-tiled +
        5 fp32 matmuls for the "+f" part (issued early), stripe-reduce, bias.
DMA triggers cost ~650ns of queue each and serialize on 8 completion
lanes, so inputs are packed and rearranges merged (~17 triggers total).
"""

from contextlib import ExitStack

import numpy as np
import ml_dtypes

import concourse.bacc as bacc
import concourse.bass as bass
import concourse.tile as tile
from concourse import mybir
from concourse import bass_utils

R, B, T, H = 4, 512, 150, 32
K = R * T                      # 600
NCORES = 8
BL = B // NCORES               # 64 batches per core
KTS = [(0, 128), (128, 128), (256, 128), (384, 128), (512, 88)]
F32 = mybir.dt.float32
BF16 = mybir.dt.bfloat16
BF = ml_dtypes.bfloat16

NDUM_HEAD = 5                  # PE warmup dummies before stage 1
NDUM_MID = 6                   # dummies bridging the Vt-DMA wait

_CACHE = {}


def build_nc():
    nc = bacc.Bacc("TRN2", target_bir_lowering=False)
    ba_d = nc.dram_tensor("ba", [128, 960], BF16, kind="ExternalInput")
    bf_d = nc.dram_tensor("bf", [128, 300], BF16, kind="ExternalInput")
    bb_d = nc.dram_tensor("bb", [128, 1405], BF16, kind="ExternalInput")
    pf_d = nc.dram_tensor("pf", [R, BL * T], BF16, kind="ExternalInput")
    zt_d = nc.dram_tensor("zt", [12, BL * 128], BF16, kind="ExternalInput")
    fp_d = nc.dram_tensor("fp", [128, 203], F32, kind="ExternalInput")
    out_d = nc.dram_tensor("out", [7, BL], F32, kind="ExternalOutput")

    with tile.TileContext(nc) as tc, ExitStack() as ctx:
        consts = ctx.enter_context(tc.tile_pool(name="consts", bufs=1))
        psA = ctx.enter_context(tc.tile_pool(name="psA", bufs=1, space="PSUM"))
        psPre = ctx.enter_context(tc.tile_pool(name="psPre", bufs=4, space="PSUM"))
        psOut = ctx.enter_context(tc.tile_pool(name="psOut", bufs=1, space="PSUM"))
        psW = ctx.enter_context(tc.tile_pool(name="psW", bufs=1, space="PSUM"))

        ba_sb = consts.tile([128, 960], BF16)
        bf_sb = consts.tile([128, 300], BF16)
        bb_sb = consts.tile([128, 1405], BF16)
        fp_sb = consts.tile([128, 203], F32)
        fn2_sb = consts.tile([128, 5, BL], BF16)
        fn3_sb = consts.tile([128, 5, BL], BF16)
        fn5_sb = consts.tile([128, 5, BL], BF16)
        f2_sb = consts.tile([128, 300], BF16)
        f3_sb = consts.tile([128, 300], BF16)
        f5_sb = consts.tile([128, 300], BF16)
        fp3_sb = consts.tile([128, 300], BF16)
        fp5_sb = consts.tile([128, 300], BF16)
        P_sb = consts.tile([12, BL, T], BF16)
        Vt = consts.tile([12, BL, 128], BF16)
        m_sb = consts.tile([128, 256], BF16)
        hm = consts.tile([128, BL, T], BF16)
        str_sb = consts.tile([128, BL], F32)
        ob = consts.tile([7, BL], F32)
        scrW = consts.tile([128, 576], BF16)

        fn_v = ba_sb[:, 0:320].rearrange("p (k b) -> p k b", k=5)
        wc_v = ba_sb[:, 320:960].rearrange("p (k c) -> p k c", k=5)
        f_v = bf_sb[:]
        ft_v = bb_sb[:, 0:320].rearrange("p (k b) -> p k b", k=5)
        wx_v = bb_sb[:, 320:355].rearrange("p (k i) -> p k i", k=5)
        u_v = bb_sb[:, 355:1405].rearrange("p (t i) -> p t i", t=T)
        kv_v = fp_sb[:, 0:3]
        sel_v = fp_sb[:, 3:10]
        bx_v = fp_sb[0:7, 10:11]
        ones_v = fp_sb[0:1, 11:75]
        wk_v = fp_sb[0:1, 75:203]

        # ---- input loads. fn|wc first (stage-1 critical), then f.
        nc.sync.dma_start(out=ba_sb[:], in_=ba_d[:])
        nc.sync.dma_start(out=bf_sb[:], in_=bf_d[:])
        nc.sync.dma_start(out=Vt[:].rearrange("q b c -> q (b c)"), in_=zt_d[:])
        nc.scalar.dma_start(out=fp_sb[:], in_=fp_d[:])
        nc.scalar.dma_start(
            out=P_sb[0:4, :, :].rearrange("q b t -> q (b t)"), in_=pf_d[:]
        )
        nc.scalar.dma_start(out=bb_sb[:], in_=bb_d[:])

        # ---- PE warmup: full-128-row matmuls (small ones don't count as
        # HAM activity); bridges the head so stage-1 starts closer to warm.
        nc.vector.memset(scrW[:], 0.0)
        nc.vector.memset(str_sb[:], 0.0)
        warm_ps = psW.tile([64, 512], F32)
        for i in range(NDUM_HEAD):
            nc.tensor.matmul(
                out=warm_ps[0:64, :],
                lhsT=scrW[:, 0:64],
                rhs=scrW[:, 64:576],
                start=True,
                stop=True,
                skip_group_check=True,
            )

        # ---- DVE powers: fn first (feeds stage-1), then f (feeds P rows)
        nc.vector.tensor_mul(fn2_sb[:], fn_v, fn_v)
        nc.vector.tensor_mul(fn3_sb[:], fn2_sb[:], fn_v)
        nc.vector.tensor_mul(fn5_sb[:], fn3_sb[:], fn2_sb[:])
        nc.vector.tensor_mul(f2_sb[:], f_v, f_v)
        nc.vector.tensor_mul(f3_sb[:], f2_sb[:], f_v)
        nc.vector.tensor_scalar_mul(out=fp3_sb[:], in0=f3_sb[:], scalar1=kv_v[:, 1:2])
        nc.vector.tensor_mul(f5_sb[:], f3_sb[:], f2_sb[:])
        nc.vector.tensor_scalar_mul(out=fp5_sb[:], in0=f5_sb[:], scalar1=kv_v[:, 2:3])

        # ---- stage 1: M_j[p, (r,c)] = fn^j @ Wc.T, col-tiled chains
        mps = psA.tile([128, 256], F32, padded_shape=[None, 512])
        for kt, (k0, kp) in enumerate(KTS):
            nc.tensor.matmul(
                out=mps[0:64, 0:128],
                lhsT=fn_v[0:kp, kt, :],
                rhs=wc_v[0:kp, kt, :],
                start=(kt == 0),
                stop=False,
                tile_position=(0, 0),
                skip_group_check=True,
            )
            nc.tensor.matmul(
                out=mps[64:128, 0:128],
                lhsT=fn3_sb[0:kp, kt, :],
                rhs=wc_v[0:kp, kt, :],
                start=(kt == 0),
                stop=(kt == 4),
                tile_position=(0, 64),
                skip_group_check=True,
            )
        nc.tensor.matmul(
            out=mps[0:64, 0:128],
            lhsT=ones_v,
            rhs=wk_v,
            start=False,
            stop=True,
            tile_position=(0, 0),
            skip_group_check=True,
        )
        # copy-A (j1+j3) fires before the j5 chain so Vt rows 0-7 DMA early
        nc.scalar.activation(
            out=m_sb[:, 0:128],
            in_=mps[:, 0:128],
            func=mybir.ActivationFunctionType.Copy,
        )
        for kt, (k0, kp) in enumerate(KTS):
            nc.tensor.matmul(
                out=mps[0:64, 128:256],
                lhsT=fn5_sb[0:kp, kt, :],
                rhs=wc_v[0:kp, kt, :],
                start=(kt == 0),
                stop=(kt == 4),
                tile_position=(0, 0),
                skip_group_check=True,
            )
        nc.vector.tensor_copy(m_sb[0:64, 128:256], mps[0:64, 128:256])

        # ---- P rows 4-11: collapse [128 part (r,h), 300 (l,t)] -> [4,64,150]
        nc.gpsimd.dma_start(out=P_sb[4:8, :, :], in_=fp3_sb[:])
        nc.gpsimd.dma_start(out=P_sb[8:12, :, :], in_=fp5_sb[:])
        # ---- Vt M-blocks: collapse [64 part, 32] -> [1, 64, 32], j-major
        m_slices = [
            lambda r: m_sb[0:64, r * H : (r + 1) * H],
            lambda r: m_sb[64:128, r * H : (r + 1) * H],
            lambda r: m_sb[0:64, 128 + r * H : 128 + (r + 1) * H],
        ]
        qs = [nc.sync, nc.scalar]
        vt_eng = [nc.sync, nc.scalar, nc.sync, nc.scalar,
                  nc.sync, nc.scalar, nc.sync, nc.scalar,
                  nc.gpsimd, nc.gpsimd, nc.sync, nc.scalar]
        for j in range(3):
            for r in range(R):
                q = j * 4 + r
                vt_eng[q].dma_start(
                    out=Vt[q : q + 1, :, r * H : (r + 1) * H], in_=m_slices[j](r)
                )

        # ---- stage 2: per b one [12,128]^T @ [12,150] matmul.
        relu_engs = [nc.scalar, nc.vector]
        pre = None
        relu_idx = 0
        for b in range(BL):
            if b % 3 == 0:
                pre = psPre.tile([128, 512], F32, name=f"pre_{b}", tag="pre")
            slot = b % 3
            nc.tensor.matmul(
                out=pre[:, slot * T : (slot + 1) * T],
                lhsT=Vt[0:12, b, :],
                rhs=P_sb[0:12, b, :],
                start=True,
                stop=True,
                tile_position=(0, 0),
                skip_group_check=True,
            )
            if b % 3 == 2 or b == BL - 1:
                nb = b % 3 + 1
                c0 = b - nb + 1
                eng = relu_engs[relu_idx % 2]
                relu_idx += 1
                dst = hm[:, c0 : c0 + nb, :]
                src = pre[:, 0 : nb * T]
                if eng is nc.scalar:
                    eng.activation(
                        out=dst, in_=src, func=mybir.ActivationFunctionType.Relu
                    )
                else:
                    eng.tensor_scalar_max(out=dst, in0=src, scalar1=0.0)

        # ---- final pass: out[i, p] accumulation, 4-way col-tiled over t.
        # stripe-0 group: t=0 starts, then the 5 fp32 "+f" matmuls (early,
        # they only need ft/wx), then the remaining t's; t=148 stops.
        op = psOut.tile([128, BL], F32, padded_shape=[None, 512])
        last_t = [148, 149, 146, 147]

        def u_mm(t):
            j4 = t % 4
            nc.tensor.matmul(
                out=op[32 * j4 : 32 * j4 + 7, 0:BL],
                lhsT=u_v[:, t, :],
                rhs=hm[:, :, t],
                start=(t == j4) and j4 > 0,
                stop=(t == last_t[j4]) if j4 > 0 else (t == 148),
                tile_position=(0, 32 * j4),
                skip_group_check=True,
            )

        for kt, (k0, kp) in enumerate(KTS):
            nc.tensor.matmul(
                out=op[0:7, 0:BL],
                lhsT=wx_v[0:kp, kt, :],
                rhs=ft_v[0:kp, kt, :],
                start=(kt == 0),
                stop=False,
                tile_position=(0, 0),
                skip_group_check=True,
            )
        for t in range(0, T):
            u_mm(t)
        # collect the 4 stripes into str_sb (zeroed), reduce with sel, add bias
        for j4 in range(4):
            if j4 % 2 == 0:
                nc.vector.tensor_copy(
                    str_sb[32 * j4 : 32 * j4 + 7, :], op[32 * j4 : 32 * j4 + 7, 0:BL]
                )
            else:
                nc.scalar.activation(
                    out=str_sb[32 * j4 : 32 * j4 + 7, :],
                    in_=op[32 * j4 : 32 * j4 + 7, 0:BL],
                    func=mybir.ActivationFunctionType.Copy,
                )
        out2 = psOut.tile([7, BL], F32, padded_shape=[None, 512])
        nc.tensor.matmul(
            out=out2[0:7, 0:BL],
            lhsT=sel_v,
            rhs=str_sb[:],
            start=True,
            stop=True,
        )
        nc.vector.tensor_scalar_add(out=ob[:], in0=out2[0:7, 0:BL], scalar1=bx_v)
        nc.sync.dma_start(out=out_d[:], in_=ob[:])

    nc.finalize()
    return nc


def _fit_coeffs(a, f, fn):
    """Per-rep LSQ fit of tanh(z) on basis (z, z^3, z^5) over the empirical
    distribution of z = a_r*f[r,b,t]*fn[b,k] (deterministic subsample)."""
    coeffs = np.zeros((R, 3), np.float64)
    fn_s = fn.ravel()[::157].astype(np.float64)
    for r in range(R):
        s_s = (float(a[r]) * f[r]).ravel()[::38].astype(np.float64)
        z = np.outer(s_s, fn_s).ravel()
        A = np.stack([z, z**3, z**5], axis=1)
        c, *_ = np.linalg.lstsq(A, np.tanh(z), rcond=None)
        coeffs[r] = c
    return coeffs


def _host_prep(feats, a, W, Wc, Wh, W1, b1, W2, b2):
    f = feats[:, :, 0, :]                              # [R,B,T]
    feat_n = feats.reshape(B, K)                       # [B,K]
    Wx = W2 @ W1                                       # [7,K]
    bx = (W2 @ b1 + b2).astype(np.float32)

    co = _fit_coeffs(a, f, feat_n)
    a64 = a.astype(np.float64)
    k1 = co[:, 0] * a64
    # clamp so wk = W/k1 stays finite; k1*f ~ 0 then, and term -> f*W exactly
    k1 = np.where(np.abs(k1) < 1e-20, 1e-20, k1)
    k3 = (co[:, 1] * a64**3).astype(np.float32)
    k5 = (co[:, 2] * a64**5).astype(np.float32)
    wk = (W / k1[:, None]).astype(np.float32)          # [R, H]
    k1 = k1.astype(np.float32)

    # ---- shared packed constants
    wc_pack = np.zeros((128, 5, 128), np.float32)
    for kt, (k0, kp) in enumerate(KTS):
        for r in range(R):
            wc_pack[:kp, kt, r * H : (r + 1) * H] = Wc[r, :, k0 : k0 + kp].T

    U = np.zeros((128, T, 7), np.float32)              # Wh[r,c]*Wx[i, r*T+t]
    for r in range(R):
        blk = Wx[:, r * T : (r + 1) * T].T             # [T,7]
        U[r * H : (r + 1) * H] = Wh[r][:, None, None] * blk[None]

    wx_pack = np.zeros((128, 5, 7), np.float32)
    for kt, (k0, kp) in enumerate(KTS):
        wx_pack[:kp, kt, :] = Wx[:, k0 : k0 + kp].T

    fp_c = np.zeros((128, 203), np.float32)
    for r in range(R):
        fp_c[r * 32 : (r + 1) * 32, 1] = k3[r]
        fp_c[r * 32 : (r + 1) * 32, 2] = k5[r]
    for j4 in range(4):
        for i in range(7):
            fp_c[32 * j4 + i, 3 + i] = 1.0             # sel
    fp_c[0:7, 10] = bx
    fp_c[0, 11:75] = 1.0                               # ones row
    fp_c[0, 75:203] = wk.reshape(128)                  # wk row

    bb_base = np.zeros((128, 1405), np.float32)
    bb_base[:, 320:355] = wx_pack.reshape(128, 35)
    bb_base[:, 355:1405] = U.reshape(128, 1050)

    zt_c = np.zeros((12, BL * 128), BF)

    fT_full = np.concatenate([f[r].T for r in range(R)], axis=0)  # [K, B]

    in_maps = []
    for m in range(NCORES):
        b0 = m * BL
        ba_h = np.zeros((128, 960), np.float32)
        for kt, (k0, kp) in enumerate(KTS):
            ba_h[:kp, 64 * kt : 64 * (kt + 1)] = feat_n[b0 : b0 + BL, k0 : k0 + kp].T
        ba_h[:, 320:960] = wc_pack.reshape(128, 640)

        # f wide: [r*32 + b//2, (b%2)*150 + t] = f[r, b0+b, t]
        bf_h = f[:, b0 : b0 + BL, :].reshape(128, 300)

        # P rows 0-3 = k1*f in [r, (b, t)] layout
        pf_h = (k1[:, None, None] * f[:, b0 : b0 + BL, :]).reshape(R, BL * T)

        bb_h = bb_base.copy()
        for kt, (k0, kp) in enumerate(KTS):
            bb_h[:kp, 64 * kt : 64 * (kt + 1)] = fT_full[k0 : k0 + kp, b0 : b0 + BL]

        in_maps.append(
            {
                "ba": ba_h.astype(BF),
                "bf": bf_h.astype(BF),
                "bb": bb_h.astype(BF),
                "pf": pf_h.astype(BF),
                "zt": zt_c,
                "fp": fp_c,
            }
        )
    return in_maps


def kernel(feats_list, a, W, Wc, Wh, W1, b1, W2, b2):
    feats = np.asarray(feats_list, np.float32)
    in_maps = _host_prep(
        feats,
        np.asarray(a, np.float32),
        np.asarray(W, np.float32),
        np.asarray(Wc, np.float32),
        np.asarray(Wh, np.float32),
        np.asarray(W1, np.float32),
        np.asarray(b1, np.float32),
        np.asarray(W2, np.float32),
        np.asarray(b2, np.float32),
    )
    if "nc" not in _CACHE:
        _CACHE["nc"] = build_nc()
    res = bass_utils.run_bass_kernel_spmd(
        _CACHE["nc"], in_maps, core_ids=list(range(NCORES))
    )
    _CACHE["last_result"] = res
    out = np.concatenate([r["out"].T for r in res.results], axis=0)  # [B,7]
    return out[:, None, :].astype(np.float32)


# revision 18
# speedup vs baseline: 1.1011x; 1.1011x over previous
"""Trainium2 Bass kernel for nn_CAM_62852551409742.

Math (reference):
  f = feats[:, :, 0, :]                               [R,B,T], R=4, B=512, T=150
  feat_n = feats.reshape(B, K)                        [B,K], K=600
  att[r,b,t,k] = tanh(a[r]*f[r,b,t] * feat_n[b,k])
  Hm = relu(att @ Wc[r].T + f*W[r])                   [R,B,T,32]
  attf = Hm @ Wh[r] + f                               [R,B,T]
  out = (ff @ W1.T + b1) @ W2.T + b2                  [B,1,7]

Key optimization: tanh of a *product* admits an odd-polynomial fit
tanh(z) ~= c1 z + c3 z^3 + c5 z^5 (runtime-LSQ-fit per rep on the actual
z distribution; rel err ~1e-6), which factorizes through the k-contraction:
  sum_k tanh(s*fn_k) Wc[c,k] = sum_j c_j s^j M_j[c],  M_j = fn^j @ Wc.T
so the 184M-element tanh tensor is never materialized. The f*W term is
folded into the M1 chain via wk = W/k1 (P row carries k1*f; k1 clamped).

Device work per core (B sharded 64/core, p == local batch):
  stage 1: M_j[p,(r,c)] = fn^j @ Wc.T (+ ones x wk fold into M1), col-tiled
           chains: j1@[0:64,0:128], j3@[64:128,0:128] (pos (0,64)),
           j5@[0:64,128:256].
  build Vt[q=j*4+r, b, (r,c)-block] block-diagonal [12,128] lhsT tiles:
        zero-filled by one DRAM DMA, 12 collapse-DMAs for the M blocks.
        P [q, b, t]: rows 0-3 streamed from DRAM (host-side k1*f), rows
        4-11 collapse-DMAs from DVE-computed k3*f^3, k5*f^5.
  stage 2: per b one [12,128]^T @ [12,150] matmul -> pre [128, 150];
           relu (DVE/ACT alternate, paces psum recycling) -> hm bf16
  final: U-trick (U[(rc),t,i] = Wh*Wx) 150 matmuls 4-way col-tiled +
        5 fp32 matmuls for the "+f" part (issued early), stripe-reduce, bias.
DMA triggers cost ~650ns of queue each and serialize on 8 completion
lanes, so inputs are packed and rearranges merged (~17 triggers total).
"""

from contextlib import ExitStack

import numpy as np
import ml_dtypes

import concourse.bacc as bacc
import concourse.bass as bass
import concourse.tile as tile
from concourse import mybir
from concourse import bass_utils

R, B, T, H = 4, 512, 150, 32
K = R * T                      # 600
NCORES = 8
BL = B // NCORES               # 64 batches per core
KTS = [(0, 128), (128, 128), (256, 128), (384, 128), (512, 88)]
F32 = mybir.dt.float32
BF16 = mybir.dt.bfloat16
BF = ml_dtypes.bfloat16

NDUM_HEAD = 5                  # PE warmup dummies before stage 1
NDUM_MID = 6                   # dummies bridging the Vt-DMA wait

_CACHE = {}


def build_nc():
    nc = bacc.Bacc("TRN2", target_bir_lowering=False)
    ba_d = nc.dram_tensor("ba", [128, 960], BF16, kind="ExternalInput")
    bf_d = nc.dram_tensor("bf", [128, 300], BF16, kind="ExternalInput")
    bb_d = nc.dram_tensor("bb", [128, 1405], BF16, kind="ExternalInput")
    pf_d = nc.dram_tensor("pf", [R, BL * T], BF16, kind="ExternalInput")
    zt_d = nc.dram_tensor("zt", [12, BL * 128], BF16, kind="ExternalInput")
    fp_d = nc.dram_tensor("fp", [128, 203], F32, kind="ExternalInput")
    out_d = nc.dram_tensor("out", [7, BL], F32, kind="ExternalOutput")

    with tile.TileContext(nc) as tc, ExitStack() as ctx:
        consts = ctx.enter_context(tc.tile_pool(name="consts", bufs=1))
        psA = ctx.enter_context(tc.tile_pool(name="psA", bufs=1, space="PSUM"))
        psPre = ctx.enter_context(tc.tile_pool(name="psPre", bufs=4, space="PSUM"))
        psOut = ctx.enter_context(tc.tile_pool(name="psOut", bufs=1, space="PSUM"))
        psW = ctx.enter_context(tc.tile_pool(name="psW", bufs=1, space="PSUM"))

        ba_sb = consts.tile([128, 960], BF16)
        bf_sb = consts.tile([128, 300], BF16)
        bb_sb = consts.tile([128, 1405], BF16)
        fp_sb = consts.tile([128, 203], F32)
        fn2_sb = consts.tile([128, 5, BL], BF16)
        fn3_sb = consts.tile([128, 5, BL], BF16)
        fn5_sb = consts.tile([128, 5, BL], BF16)
        f2_sb = consts.tile([128, 300], BF16)
        f3_sb = consts.tile([128, 300], BF16)
        f5_sb = consts.tile([128, 300], BF16)
        fp3_sb = consts.tile([128, 300], BF16)
        fp5_sb = consts.tile([128, 300], BF16)
        P_sb = consts.tile([12, BL, T], BF16)
        Vt = consts.tile([12, BL, 128], BF16)
        m_sb = consts.tile([128, 256], BF16)
        hm = consts.tile([128, BL, T], BF16)
        str_sb = consts.tile([128, BL], F32)
        ob = consts.tile([7, BL], F32)
        scrW = consts.tile([128, 576], BF16)

        fn_v = ba_sb[:, 0:320].rearrange("p (k b) -> p k b", k=5)
        wc_v = ba_sb[:, 320:960].rearrange("p (k c) -> p k c", k=5)
        f_v = bf_sb[:]
        ft_v = bb_sb[:, 0:320].rearrange("p (k b) -> p k b", k=5)
        wx_v = bb_sb[:, 320:355].rearrange("p (k i) -> p k i", k=5)
        u_v = bb_sb[:, 355:1405].rearrange("p (t i) -> p t i", t=T)
        kv_v = fp_sb[:, 0:3]
        sel_v = fp_sb[:, 3:10]
        bx_v = fp_sb[0:7, 10:11]
        ones_v = fp_sb[0:1, 11:75]
        wk_v = fp_sb[0:1, 75:203]

        # ---- input loads. fn|wc first (stage-1 critical), then f.
        nc.sync.dma_start(out=ba_sb[:], in_=ba_d[:])
        nc.sync.dma_start(out=bf_sb[:], in_=bf_d[:])
        nc.sync.dma_start(out=Vt[:].rearrange("q b c -> q (b c)"), in_=zt_d[:])
        nc.scalar.dma_start(out=fp_sb[:], in_=fp_d[:])
        nc.scalar.dma_start(
            out=P_sb[0:4, :, :].rearrange("q b t -> q (b t)"), in_=pf_d[:]
        )
        nc.scalar.dma_start(out=bb_sb[:], in_=bb_d[:])

        # ---- PE warmup: full-128-row matmuls (small ones don't count as
        # HAM activity); bridges the head so stage-1 starts closer to warm.
        nc.vector.memset(scrW[:], 0.0)
        nc.vector.memset(str_sb[:], 0.0)
        warm_ps = psW.tile([64, 512], F32)
        for i in range(NDUM_HEAD):
            nc.tensor.matmul(
                out=warm_ps[0:64, :],
                lhsT=scrW[:, 0:64],
                rhs=scrW[:, 64:576],
                start=True,
                stop=True,
                skip_group_check=True,
            )

        # ---- DVE powers: fn first (feeds stage-1), then f (feeds P rows)
        nc.vector.tensor_mul(fn2_sb[:], fn_v, fn_v)
        nc.vector.tensor_mul(fn3_sb[:], fn2_sb[:], fn_v)
        nc.vector.tensor_mul(fn5_sb[:], fn3_sb[:], fn2_sb[:])
        nc.vector.tensor_mul(f2_sb[:], f_v, f_v)
        nc.vector.tensor_mul(f3_sb[:], f2_sb[:], f_v)
        nc.vector.tensor_scalar_mul(out=fp3_sb[:], in0=f3_sb[:], scalar1=kv_v[:, 1:2])
        nc.vector.tensor_mul(f5_sb[:], f3_sb[:], f2_sb[:])
        nc.vector.tensor_scalar_mul(out=fp5_sb[:], in0=f5_sb[:], scalar1=kv_v[:, 2:3])

        # ---- stage 1: M_j[p, (r,c)] = fn^j @ Wc.T, col-tiled chains
        mps = psA.tile([128, 256], F32, padded_shape=[None, 512])
        for kt, (k0, kp) in enumerate(KTS):
            nc.tensor.matmul(
                out=mps[0:64, 0:128],
                lhsT=fn_v[0:kp, kt, :],
                rhs=wc_v[0:kp, kt, :],
                start=(kt == 0),
                stop=False,
                tile_position=(0, 0),
                skip_group_check=True,
            )
            nc.tensor.matmul(
                out=mps[64:128, 0:128],
                lhsT=fn3_sb[0:kp, kt, :],
                rhs=wc_v[0:kp, kt, :],
                start=(kt == 0),
                stop=(kt == 4),
                tile_position=(0, 64),
                skip_group_check=True,
            )
        nc.tensor.matmul(
            out=mps[0:64, 0:128],
            lhsT=ones_v,
            rhs=wk_v,
            start=False,
            stop=True,
            tile_position=(0, 0),
            skip_group_check=True,
        )
        for kt, (k0, kp) in enumerate(KTS):
            nc.tensor.matmul(
                out=mps[64:128, 128:256],
                lhsT=fn5_sb[0:kp, kt, :],
                rhs=wc_v[0:kp, kt, :],
                start=(kt == 0),
                stop=(kt == 4),
                tile_position=(0, 64),
                skip_group_check=True,
            )

        # single psum->bf16 copy for all three chains (ACT)
        nc.scalar.activation(
            out=m_sb[:],
            in_=mps[:],
            func=mybir.ActivationFunctionType.Copy,
        )

        # ---- P rows 4-11: collapse [128 part (r,h), 300 (l,t)] -> [4,64,150]
        nc.gpsimd.dma_start(out=P_sb[4:8, :, :], in_=fp3_sb[:])
        nc.gpsimd.dma_start(out=P_sb[8:12, :, :], in_=fp5_sb[:])
        # ---- Vt M-blocks: collapse [64 part, 32] -> [1, 64, 32], j-major
        m_slices = [
            lambda r: m_sb[0:64, r * H : (r + 1) * H],
            lambda r: m_sb[64:128, r * H : (r + 1) * H],
            lambda r: m_sb[64:128, 128 + r * H : 128 + (r + 1) * H],
        ]
        qs = [nc.sync, nc.scalar]
        vt_eng = [nc.sync, nc.scalar, nc.sync, nc.scalar,
                  nc.sync, nc.scalar, nc.sync, nc.scalar,
                  nc.gpsimd, nc.gpsimd, nc.sync, nc.scalar]
        for j in range(3):
            for r in range(R):
                q = j * 4 + r
                vt_eng[q].dma_start(
                    out=Vt[q : q + 1, :, r * H : (r + 1) * H], in_=m_slices[j](r)
                )

        # ---- stage 2: per b one [12,128]^T @ [12,150] matmul.
        relu_engs = [nc.scalar, nc.vector]
        pre = None
        relu_idx = 0
        for b in range(BL):
            if b % 3 == 0:
                pre = psPre.tile([128, 512], F32, name=f"pre_{b}", tag="pre")
            slot = b % 3
            nc.tensor.matmul(
                out=pre[:, slot * T : (slot + 1) * T],
                lhsT=Vt[0:12, b, :],
                rhs=P_sb[0:12, b, :],
                start=True,
                stop=True,
                tile_position=(0, 0),
                skip_group_check=True,
            )
            if b % 3 == 2 or b == BL - 1:
                nb = b % 3 + 1
                c0 = b - nb + 1
                eng = relu_engs[relu_idx % 2]
                relu_idx += 1
                dst = hm[:, c0 : c0 + nb, :]
                src = pre[:, 0 : nb * T]
                if eng is nc.scalar:
                    eng.activation(
                        out=dst, in_=src, func=mybir.ActivationFunctionType.Relu
                    )
                else:
                    eng.tensor_scalar_max(out=dst, in0=src, scalar1=0.0)

        # ---- final pass: out[i, p] accumulation, 4-way col-tiled over t.
        # stripe-0 group: t=0 starts, then the 5 fp32 "+f" matmuls (early,
        # they only need ft/wx), then the remaining t's; t=148 stops.
        op = psOut.tile([128, BL], F32, padded_shape=[None, 512])
        last_t = [148, 149, 146, 147]

        def u_mm(t):
            j4 = t % 4
            nc.tensor.matmul(
                out=op[32 * j4 : 32 * j4 + 7, 0:BL],
                lhsT=u_v[:, t, :],
                rhs=hm[:, :, t],
                start=(t == j4) and j4 > 0,
                stop=(t == last_t[j4]) if j4 > 0 else (t == 148),
                tile_position=(0, 32 * j4),
                skip_group_check=True,
            )

        for kt, (k0, kp) in enumerate(KTS):
            nc.tensor.matmul(
                out=op[0:7, 0:BL],
                lhsT=wx_v[0:kp, kt, :],
                rhs=ft_v[0:kp, kt, :],
                start=(kt == 0),
                stop=False,
                tile_position=(0, 0),
                skip_group_check=True,
            )
        for t in range(0, T):
            u_mm(t)
        # collect the 4 stripes into str_sb (zeroed), reduce with sel, add bias
        for j4 in range(4):
            if j4 % 2 == 0:
                nc.vector.tensor_copy(
                    str_sb[32 * j4 : 32 * j4 + 7, :], op[32 * j4 : 32 * j4 + 7, 0:BL]
                )
            else:
                nc.scalar.activation(
                    out=str_sb[32 * j4 : 32 * j4 + 7, :],
                    in_=op[32 * j4 : 32 * j4 + 7, 0:BL],
                    func=mybir.ActivationFunctionType.Copy,
                )
        out2 = psOut.tile([7, BL], F32, padded_shape=[None, 512])
        nc.tensor.matmul(
            out=out2[0:7, 0:BL],
            lhsT=sel_v,
            rhs=str_sb[:],
            start=True,
            stop=True,
        )
        nc.vector.tensor_scalar_add(out=ob[:], in0=out2[0:7, 0:BL], scalar1=bx_v)
        nc.sync.dma_start(out=out_d[:], in_=ob[:])

    nc.finalize()
    return nc


def _fit_coeffs(a, f, fn):
    """Per-rep LSQ fit of tanh(z) on basis (z, z^3, z^5) over the empirical
    distribution of z = a_r*f[r,b,t]*fn[b,k] (deterministic subsample)."""
    coeffs = np.zeros((R, 3), np.float64)
    fn_s = fn.ravel()[::157].astype(np.float64)
    for r in range(R):
        s_s = (float(a[r]) * f[r]).ravel()[::38].astype(np.float64)
        z = np.outer(s_s, fn_s).ravel()
        A = np.stack([z, z**3, z**5], axis=1)
        c, *_ = np.linalg.lstsq(A, np.tanh(z), rcond=None)
        coeffs[r] = c
    return coeffs


def _host_prep(feats, a, W, Wc, Wh, W1, b1, W2, b2):
    f = feats[:, :, 0, :]                              # [R,B,T]
    feat_n = feats.reshape(B, K)                       # [B,K]
    Wx = W2 @ W1                                       # [7,K]
    bx = (W2 @ b1 + b2).astype(np.float32)

    co = _fit_coeffs(a, f, feat_n)
    a64 = a.astype(np.float64)
    k1 = co[:, 0] * a64
    # clamp so wk = W/k1 stays finite; k1*f ~ 0 then, and term -> f*W exactly
    k1 = np.where(np.abs(k1) < 1e-20, 1e-20, k1)
    k3 = (co[:, 1] * a64**3).astype(np.float32)
    k5 = (co[:, 2] * a64**5).astype(np.float32)
    wk = (W / k1[:, None]).astype(np.float32)          # [R, H]
    k1 = k1.astype(np.float32)

    # ---- shared packed constants
    wc_pack = np.zeros((128, 5, 128), np.float32)
    for kt, (k0, kp) in enumerate(KTS):
        for r in range(R):
            wc_pack[:kp, kt, r * H : (r + 1) * H] = Wc[r, :, k0 : k0 + kp].T

    U = np.zeros((128, T, 7), np.float32)              # Wh[r,c]*Wx[i, r*T+t]
    for r in range(R):
        blk = Wx[:, r * T : (r + 1) * T].T             # [T,7]
        U[r * H : (r + 1) * H] = Wh[r][:, None, None] * blk[None]

    wx_pack = np.zeros((128, 5, 7), np.float32)
    for kt, (k0, kp) in enumerate(KTS):
        wx_pack[:kp, kt, :] = Wx[:, k0 : k0 + kp].T

    fp_c = np.zeros((128, 203), np.float32)
    for r in range(R):
        fp_c[r * 32 : (r + 1) * 32, 1] = k3[r]
        fp_c[r * 32 : (r + 1) * 32, 2] = k5[r]
    for j4 in range(4):
        for i in range(7):
            fp_c[32 * j4 + i, 3 + i] = 1.0             # sel
    fp_c[0:7, 10] = bx
    fp_c[0, 11:75] = 1.0                               # ones row
    fp_c[0, 75:203] = wk.reshape(128)                  # wk row

    bb_base = np.zeros((128, 1405), np.float32)
    bb_base[:, 320:355] = wx_pack.reshape(128, 35)
    bb_base[:, 355:1405] = U.reshape(128, 1050)

    zt_c = np.zeros((12, BL * 128), BF)

    fT_full = np.concatenate([f[r].T for r in range(R)], axis=0)  # [K, B]

    in_maps = []
    for m in range(NCORES):
        b0 = m * BL
        ba_h = np.zeros((128, 960), np.float32)
        for kt, (k0, kp) in enumerate(KTS):
            ba_h[:kp, 64 * kt : 64 * (kt + 1)] = feat_n[b0 : b0 + BL, k0 : k0 + kp].T
        ba_h[:, 320:960] = wc_pack.reshape(128, 640)

        # f wide: [r*32 + b//2, (b%2)*150 + t] = f[r, b0+b, t]
        bf_h = f[:, b0 : b0 + BL, :].reshape(128, 300)

        # P rows 0-3 = k1*f in [r, (b, t)] layout
        pf_h = (k1[:, None, None] * f[:, b0 : b0 + BL, :]).reshape(R, BL * T)

        bb_h = bb_base.copy()
        for kt, (k0, kp) in enumerate(KTS):
            bb_h[:kp, 64 * kt : 64 * (kt + 1)] = fT_full[k0 : k0 + kp, b0 : b0 + BL]

        in_maps.append(
            {
                "ba": ba_h.astype(BF),
                "bf": bf_h.astype(BF),
                "bb": bb_h.astype(BF),
                "pf": pf_h.astype(BF),
                "zt": zt_c,
                "fp": fp_c,
            }
        )
    return in_maps


def kernel(feats_list, a, W, Wc, Wh, W1, b1, W2, b2):
    feats = np.asarray(feats_list, np.float32)
    in_maps = _host_prep(
        feats,
        np.asarray(a, np.float32),
        np.asarray(W, np.float32),
        np.asarray(Wc, np.float32),
        np.asarray(Wh, np.float32),
        np.asarray(W1, np.float32),
        np.asarray(b1, np.float32),
        np.asarray(W2, np.float32),
        np.asarray(b2, np.float32),
    )
    if "nc" not in _CACHE:
        _CACHE["nc"] = build_nc()
    res = bass_utils.run_bass_kernel_spmd(
        _CACHE["nc"], in_maps, core_ids=list(range(NCORES))
    )
    _CACHE["last_result"] = res
    out = np.concatenate([r["out"].T for r in res.results], axis=0)  # [B,7]
    return out[:, None, :].astype(np.float32)


# revision 20
# speedup vs baseline: 1.1694x; 1.0620x over previous
"""Trainium2 Bass kernel for nn_CAM_62852551409742.

Math (reference):
  f = feats[:, :, 0, :]                               [R,B,T], R=4, B=512, T=150
  feat_n = feats.reshape(B, K)                        [B,K], K=600
  att[r,b,t,k] = tanh(a[r]*f[r,b,t] * feat_n[b,k])
  Hm = relu(att @ Wc[r].T + f*W[r])                   [R,B,T,32]
  attf = Hm @ Wh[r] + f                               [R,B,T]
  out = (ff @ W1.T + b1) @ W2.T + b2                  [B,1,7]

Key optimization: tanh of a *product* admits an odd-polynomial fit
tanh(z) ~= c1 z + c3 z^3 + c5 z^5 (runtime-LSQ-fit per rep on the actual
z distribution; rel err ~1e-6), which factorizes through the k-contraction:
  sum_k tanh(s*fn_k) Wc[c,k] = sum_j c_j s^j M_j[c],  M_j = fn^j @ Wc.T
so the 184M-element tanh tensor is never materialized. The f*W term is
folded into the M1 chain via wk = W/k1 (P row carries k1*f; k1 clamped).

Device work per core (B sharded 64/core, p == local batch):
  stage 1: M_j[p,(r,c)] = fn^j @ Wc.T (+ ones x wk fold into M1), col-tiled
           chains: j1@[0:64,0:128], j3@[64:128,0:128] (pos (0,64)),
           j5@[0:64,128:256].
  build Vt[q=j*4+r, b, (r,c)-block] block-diagonal [12,128] lhsT tiles:
        zero-filled by one DRAM DMA, 12 collapse-DMAs for the M blocks.
        P [q, b, t]: rows 0-3 streamed from DRAM (host-side k1*f), rows
        4-11 collapse-DMAs from DVE-computed k3*f^3, k5*f^5.
  stage 2: per b one [12,128]^T @ [12,150] matmul -> pre [128, 150];
           relu (DVE/ACT alternate, paces psum recycling) -> hm bf16
  final: U-trick (U[(rc),t,i] = Wh*Wx) 150 matmuls 4-way col-tiled +
        5 fp32 matmuls for the "+f" part (issued early), stripe-reduce, bias.
DMA triggers cost ~650ns of queue each and serialize on 8 completion
lanes, so inputs are packed and rearranges merged (~17 triggers total).
"""

from contextlib import ExitStack

import numpy as np
import ml_dtypes

import concourse.bacc as bacc
import concourse.bass as bass
import concourse.tile as tile
from concourse import mybir
from concourse import bass_utils

R, B, T, H = 4, 512, 150, 32
K = R * T                      # 600
NCORES = 8
BL = B // NCORES               # 64 batches per core
KTS = [(0, 128), (128, 128), (256, 128), (384, 128), (512, 88)]
F32 = mybir.dt.float32
BF16 = mybir.dt.bfloat16
BF = ml_dtypes.bfloat16

NDUM_HEAD = 4                  # PE warmup dummies before stage 1
NDUM_MID = 6                   # dummies bridging the Vt-DMA wait

_CACHE = {}


def build_nc():
    nc = bacc.Bacc("TRN2", target_bir_lowering=False)
    ba_d = nc.dram_tensor("ba", [128, 960], BF16, kind="ExternalInput")
    bf_d = nc.dram_tensor("bf", [128, 300], BF16, kind="ExternalInput")
    bb_d = nc.dram_tensor("bb", [128, 1405], BF16, kind="ExternalInput")
    pf_d = nc.dram_tensor("pf", [R, BL * T], BF16, kind="ExternalInput")
    zt_d = nc.dram_tensor("zt", [12, BL * 128], BF16, kind="ExternalInput")
    fp_d = nc.dram_tensor("fp", [128, 203], F32, kind="ExternalInput")
    out_d = nc.dram_tensor("out", [7, BL], F32, kind="ExternalOutput")

    with tile.TileContext(nc) as tc, ExitStack() as ctx:
        consts = ctx.enter_context(tc.tile_pool(name="consts", bufs=1))
        psA = ctx.enter_context(tc.tile_pool(name="psA", bufs=1, space="PSUM"))
        psPre = ctx.enter_context(tc.tile_pool(name="psPre", bufs=4, space="PSUM"))
        psOut = ctx.enter_context(tc.tile_pool(name="psOut", bufs=1, space="PSUM"))
        psW = ctx.enter_context(tc.tile_pool(name="psW", bufs=1, space="PSUM"))

        ba_sb = consts.tile([128, 960], BF16)
        bf_sb = consts.tile([128, 300], BF16)
        bb_sb = consts.tile([128, 1405], BF16)
        fp_sb = consts.tile([128, 203], F32)
        fn2_sb = consts.tile([128, 5, BL], BF16)
        fn3_sb = consts.tile([128, 5, BL], BF16)
        fn5_sb = consts.tile([128, 5, BL], BF16)
        f2_sb = consts.tile([128, 300], BF16)
        f3_sb = consts.tile([128, 300], BF16)
        f5_sb = consts.tile([128, 300], BF16)
        fp3_sb = consts.tile([128, 300], BF16)
        fp5_sb = consts.tile([128, 300], BF16)
        P_sb = consts.tile([12, BL, T], BF16)
        Vt = consts.tile([12, BL, 128], BF16)
        m_sb = consts.tile([128, 256], BF16)
        hm = consts.tile([128, BL, T], BF16)
        str_sb = consts.tile([128, BL], F32)
        ob = consts.tile([7, BL], F32)
        scrW = consts.tile([128, 576], BF16)

        fn_v = ba_sb[:, 0:320].rearrange("p (k b) -> p k b", k=5)
        wc_v = ba_sb[:, 320:960].rearrange("p (k c) -> p k c", k=5)
        f_v = bf_sb[:]
        ft_v = bb_sb[:, 0:320].rearrange("p (k b) -> p k b", k=5)
        wx_v = bb_sb[:, 320:355].rearrange("p (k i) -> p k i", k=5)
        u_v = bb_sb[:, 355:1405].rearrange("p (t i) -> p t i", t=T)
        kv_v = fp_sb[:, 0:3]
        sel_v = fp_sb[:, 3:10]
        bx_v = fp_sb[0:7, 10:11]
        ones_v = fp_sb[0:1, 11:75]
        wk_v = fp_sb[0:1, 75:203]

        # ---- input loads. fn|wc first (stage-1 critical), then f.
        nc.sync.dma_start(out=ba_sb[:], in_=ba_d[:])
        nc.sync.dma_start(out=bf_sb[:], in_=bf_d[:])
        nc.sync.dma_start(out=Vt[:].rearrange("q b c -> q (b c)"), in_=zt_d[:])
        nc.scalar.dma_start(out=fp_sb[:], in_=fp_d[:])
        nc.scalar.dma_start(
            out=P_sb[0:4, :, :].rearrange("q b t -> q (b t)"), in_=pf_d[:]
        )
        nc.scalar.dma_start(out=bb_sb[:], in_=bb_d[:])

        # ---- PE warmup: full-128-row matmuls (small ones don't count as
        # HAM activity); bridges the head so stage-1 starts closer to warm.
        nc.vector.memset(scrW[:], 0.0)
        nc.vector.memset(str_sb[:], 0.0)
        warm_ps = psW.tile([64, 512], F32)
        for i in range(NDUM_HEAD):
            nc.tensor.matmul(
                out=warm_ps[0:64, :],
                lhsT=scrW[:, 0:64],
                rhs=scrW[:, 64:576],
                start=True,
                stop=True,
                skip_group_check=True,
            )

        # ---- DVE powers: fn first (feeds stage-1), then f (feeds P rows)
        nc.vector.tensor_mul(fn2_sb[:], fn_v, fn_v)
        nc.vector.tensor_mul(fn3_sb[:], fn2_sb[:], fn_v)
        nc.vector.tensor_mul(fn5_sb[:], fn3_sb[:], fn2_sb[:])
        nc.vector.tensor_mul(f2_sb[:], f_v, f_v)
        nc.vector.tensor_mul(f3_sb[:], f2_sb[:], f_v)
        nc.vector.tensor_scalar_mul(out=fp3_sb[:], in0=f3_sb[:], scalar1=kv_v[:, 1:2])
        nc.vector.tensor_mul(f5_sb[:], f3_sb[:], f2_sb[:])
        nc.vector.tensor_scalar_mul(out=fp5_sb[:], in0=f5_sb[:], scalar1=kv_v[:, 2:3])

        # ---- stage 1: M_j[p, (r,c)] = fn^j @ Wc.T, col-tiled chains
        mps = psA.tile([128, 256], F32, padded_shape=[None, 512])
        for kt, (k0, kp) in enumerate(KTS):
            nc.tensor.matmul(
                out=mps[0:64, 0:128],
                lhsT=fn_v[0:kp, kt, :],
                rhs=wc_v[0:kp, kt, :],
                start=(kt == 0),
                stop=False,
                tile_position=(0, 0),
                skip_group_check=True,
            )
            nc.tensor.matmul(
                out=mps[64:128, 0:128],
                lhsT=fn3_sb[0:kp, kt, :],
                rhs=wc_v[0:kp, kt, :],
                start=(kt == 0),
                stop=(kt == 4),
                tile_position=(0, 64),
                skip_group_check=True,
            )
        nc.tensor.matmul(
            out=mps[0:64, 0:128],
            lhsT=ones_v,
            rhs=wk_v,
            start=False,
            stop=True,
            tile_position=(0, 0),
            skip_group_check=True,
        )
        for kt, (k0, kp) in enumerate(KTS):
            nc.tensor.matmul(
                out=mps[64:128, 128:256],
                lhsT=fn5_sb[0:kp, kt, :],
                rhs=wc_v[0:kp, kt, :],
                start=(kt == 0),
                stop=(kt == 4),
                tile_position=(0, 64),
                skip_group_check=True,
            )

        # single psum->bf16 copy for all three chains (ACT)
        nc.scalar.activation(
            out=m_sb[:],
            in_=mps[:],
            func=mybir.ActivationFunctionType.Copy,
        )

        # ---- P rows 4-11: collapse [128 part (r,h), 300 (l,t)] -> [4,64,150]
        nc.gpsimd.dma_start(out=P_sb[4:8, :, :], in_=fp3_sb[:])
        nc.gpsimd.dma_start(out=P_sb[8:12, :, :], in_=fp5_sb[:])
        # ---- Vt M-blocks: collapse [64 part, 32] -> [1, 64, 32], j-major
        m_slices = [
            lambda r: m_sb[0:64, r * H : (r + 1) * H],
            lambda r: m_sb[64:128, r * H : (r + 1) * H],
            lambda r: m_sb[64:128, 128 + r * H : 128 + (r + 1) * H],
        ]
        qs = [nc.sync, nc.scalar]
        vt_eng = [nc.sync, nc.scalar, nc.sync, nc.scalar,
                  nc.sync, nc.scalar, nc.sync, nc.scalar,
                  nc.gpsimd, nc.gpsimd, nc.sync, nc.scalar]
        for j in range(3):
            for r in range(R):
                q = j * 4 + r
                vt_eng[q].dma_start(
                    out=Vt[q : q + 1, :, r * H : (r + 1) * H], in_=m_slices[j](r)
                )

        # ---- stage 2: per b one [12,128]^T @ [12,150] matmul.
        relu_engs = [nc.scalar, nc.vector]
        pre = None
        relu_idx = 0
        for b in range(BL):
            if b % 3 == 0:
                pre = psPre.tile([128, 512], F32, name=f"pre_{b}", tag="pre")
            slot = b % 3
            nc.tensor.matmul(
                out=pre[:, slot * T : (slot + 1) * T],
                lhsT=Vt[0:12, b, :],
                rhs=P_sb[0:12, b, :],
                start=True,
                stop=True,
                tile_position=(0, 0),
                skip_group_check=True,
            )
            if b % 3 == 2 or b == BL - 1:
                nb = b % 3 + 1
                c0 = b - nb + 1
                eng = relu_engs[relu_idx % 2]
                relu_idx += 1
                dst = hm[:, c0 : c0 + nb, :]
                src = pre[:, 0 : nb * T]
                if eng is nc.scalar:
                    eng.activation(
                        out=dst, in_=src, func=mybir.ActivationFunctionType.Relu
                    )
                else:
                    eng.tensor_scalar_max(out=dst, in0=src, scalar1=0.0)

        # ---- final pass: out[i, p] accumulation, 4-way col-tiled over t.
        # op is zeroed once so the partitions between stripes stay finite
        # and one [103, 64] copy can collect all 4 stripes.
        op = psOut.tile([128, BL], F32, padded_shape=[None, 512])
        nc.vector.memset(op[:], 0.0)
        last_t = [148, 149, 146, 147]

        def u_mm(t):
            j4 = t % 4
            nc.tensor.matmul(
                out=op[32 * j4 : 32 * j4 + 7, 0:BL],
                lhsT=u_v[:, t, :],
                rhs=hm[:, :, t],
                start=(t == j4) and j4 > 0,
                stop=(t == last_t[j4]) if j4 > 0 else (t == 148),
                tile_position=(0, 32 * j4),
                skip_group_check=True,
            )

        for kt, (k0, kp) in enumerate(KTS):
            nc.tensor.matmul(
                out=op[0:7, 0:BL],
                lhsT=wx_v[0:kp, kt, :],
                rhs=ft_v[0:kp, kt, :],
                start=(kt == 0),
                stop=False,
                tile_position=(0, 0),
                skip_group_check=True,
            )
        for t in range(0, T):
            u_mm(t)
        # collect all 4 stripes in one copy (op pre-zeroed between stripes)
        nc.vector.tensor_copy(str_sb[0:103, :], op[0:103, 0:BL])
        out2 = psOut.tile([7, BL], F32, padded_shape=[None, 512])
        nc.tensor.matmul(
            out=out2[0:7, 0:BL],
            lhsT=sel_v,
            rhs=str_sb[:],
            start=True,
            stop=True,
        )
        nc.vector.tensor_scalar_add(out=ob[:], in0=out2[0:7, 0:BL], scalar1=bx_v)
        nc.sync.dma_start(out=out_d[:], in_=ob[:])

    nc.finalize()
    return nc


def _fit_coeffs(a, f, fn):
    """Per-rep LSQ fit of tanh(z) on basis (z, z^3, z^5) over the empirical
    distribution of z = a_r*f[r,b,t]*fn[b,k] (deterministic subsample)."""
    coeffs = np.zeros((R, 3), np.float64)
    fn_s = fn.ravel()[::157].astype(np.float64)
    for r in range(R):
        s_s = (float(a[r]) * f[r]).ravel()[::38].astype(np.float64)
        z = np.outer(s_s, fn_s).ravel()
        A = np.stack([z, z**3, z**5], axis=1)
        c, *_ = np.linalg.lstsq(A, np.tanh(z), rcond=None)
        coeffs[r] = c
    return coeffs


def _host_prep(feats, a, W, Wc, Wh, W1, b1, W2, b2):
    f = feats[:, :, 0, :]                              # [R,B,T]
    feat_n = feats.reshape(B, K)                       # [B,K]
    Wx = W2 @ W1                                       # [7,K]
    bx = (W2 @ b1 + b2).astype(np.float32)

    co = _fit_coeffs(a, f, feat_n)
    a64 = a.astype(np.float64)
    k1 = co[:, 0] * a64
    # clamp so wk = W/k1 stays finite; k1*f ~ 0 then, and term -> f*W exactly
    k1 = np.where(np.abs(k1) < 1e-20, 1e-20, k1)
    k3 = (co[:, 1] * a64**3).astype(np.float32)
    k5 = (co[:, 2] * a64**5).astype(np.float32)
    wk = (W / k1[:, None]).astype(np.float32)          # [R, H]
    k1 = k1.astype(np.float32)

    # ---- shared packed constants
    wc_pack = np.zeros((128, 5, 128), np.float32)
    for kt, (k0, kp) in enumerate(KTS):
        for r in range(R):
            wc_pack[:kp, kt, r * H : (r + 1) * H] = Wc[r, :, k0 : k0 + kp].T

    U = np.zeros((128, T, 7), np.float32)              # Wh[r,c]*Wx[i, r*T+t]
    for r in range(R):
        blk = Wx[:, r * T : (r + 1) * T].T             # [T,7]
        U[r * H : (r + 1) * H] = Wh[r][:, None, None] * blk[None]

    wx_pack = np.zeros((128, 5, 7), np.float32)
    for kt, (k0, kp) in enumerate(KTS):
        wx_pack[:kp, kt, :] = Wx[:, k0 : k0 + kp].T

    fp_c = np.zeros((128, 203), np.float32)
    for r in range(R):
        fp_c[r * 32 : (r + 1) * 32, 1] = k3[r]
        fp_c[r * 32 : (r + 1) * 32, 2] = k5[r]
    for j4 in range(4):
        for i in range(7):
            fp_c[32 * j4 + i, 3 + i] = 1.0             # sel
    fp_c[0:7, 10] = bx
    fp_c[0, 11:75] = 1.0                               # ones row
    fp_c[0, 75:203] = wk.reshape(128)                  # wk row

    bb_base = np.zeros((128, 1405), np.float32)
    bb_base[:, 320:355] = wx_pack.reshape(128, 35)
    bb_base[:, 355:1405] = U.reshape(128, 1050)

    zt_c = np.zeros((12, BL * 128), BF)

    fT_full = np.concatenate([f[r].T for r in range(R)], axis=0)  # [K, B]

    in_maps = []
    for m in range(NCORES):
        b0 = m * BL
        ba_h = np.zeros((128, 960), np.float32)
        for kt, (k0, kp) in enumerate(KTS):
            ba_h[:kp, 64 * kt : 64 * (kt + 1)] = feat_n[b0 : b0 + BL, k0 : k0 + kp].T
        ba_h[:, 320:960] = wc_pack.reshape(128, 640)

        # f wide: [r*32 + b//2, (b%2)*150 + t] = f[r, b0+b, t]
        bf_h = f[:, b0 : b0 + BL, :].reshape(128, 300)

        # P rows 0-3 = k1*f in [r, (b, t)] layout
        pf_h = (k1[:, None, None] * f[:, b0 : b0 + BL, :]).reshape(R, BL * T)

        bb_h = bb_base.copy()
        for kt, (k0, kp) in enumerate(KTS):
            bb_h[:kp, 64 * kt : 64 * (kt + 1)] = fT_full[k0 : k0 + kp, b0 : b0 + BL]

        in_maps.append(
            {
                "ba": ba_h.astype(BF),
                "bf": bf_h.astype(BF),
                "bb": bb_h.astype(BF),
                "pf": pf_h.astype(BF),
                "zt": zt_c,
                "fp": fp_c,
            }
        )
    return in_maps


def kernel(feats_list, a, W, Wc, Wh, W1, b1, W2, b2):
    feats = np.asarray(feats_list, np.float32)
    in_maps = _host_prep(
        feats,
        np.asarray(a, np.float32),
        np.asarray(W, np.float32),
        np.asarray(Wc, np.float32),
        np.asarray(Wh, np.float32),
        np.asarray(W1, np.float32),
        np.asarray(b1, np.float32),
        np.asarray(W2, np.float32),
        np.asarray(b2, np.float32),
    )
    if "nc" not in _CACHE:
        _CACHE["nc"] = build_nc()
    res = bass_utils.run_bass_kernel_spmd(
        _CACHE["nc"], in_maps, core_ids=list(range(NCORES))
    )
    _CACHE["last_result"] = res
    out = np.concatenate([r["out"].T for r in res.results], axis=0)  # [B,7]
    return out[:, None, :].astype(np.float32)
